# revision 6
# baseline (speedup 1.0000x reference)
"""Trainium2 Bass kernel for a 2-layer mean-aggregation GCN + dot-product scoring.

Reference computation (per layer l in {0,1}):
    agg  = segment_sum(h[src], dst) / max(deg, 1)      # mean over incoming edges
    h    = tanh(agg @ Wl.T + bl)
Then:
    score[b, j] = sum_d h[user_index[b, j], d] * h[item_index[b, j], d]

Distribution strategy (8 NeuronCores):
  * Edges are sorted by dst on the host; each core owns a contiguous range of
    n_nodes/8 destination nodes and the edges that point into it.
  * The per-layer gather table g = h @ W.T (weight folded in: A(hW^T)=(Ah)W^T)
    is replicated in every core's DRAM via AllGather, fp32 with a 65th "ones"
    column so one matmul accumulates both feature sums and degree counts.
  * Per 128-dst tile, edges are fetched 128 at a time with indirect DMA
    (one row per partition), a one-hot [128e x 128dst] is built on DVE
    (iota == dstloc), and the segment sum accumulates in PSUM via PE matmuls:
    acc += onehot.T @ feats.
  * Mean + bias + tanh on ACT/DVE; layer-0 output is immediately multiplied by
    W1.T (PE) to form the next gather table; layer-1 output (fp32) is
    AllGathered and the final user/item rows are gathered + dotted on DVE.
"""

import numpy as np

P = 128

DEFAULT_CFG = dict(
    n_nodes=50000,
    d=64,
    n_edges=1250000,
    batch=1024,
    k=100,
    n_cores=8,
    kb=9,    # edge-tiles per one-hot DVE op
    kc=50,   # scoring chunk
)


def derived(cfg):
    nn_ = cfg["n_nodes"]
    ncores = cfg["n_cores"]
    assert nn_ % ncores == 0
    npc = nn_ // ncores              # dst nodes per core
    nt = -(-npc // P)                # dst tiles per core
    assert cfg["batch"] % ncores == 0
    rows = cfg["batch"] // ncores    # score rows per core
    assert rows <= P
    return npc, nt, rows


def preprocess(inputs, cfg):
    """Host-side sharding / index prep. Returns (in_maps, et)."""
    emb = np.asarray(inputs["embeddings"], np.float32)
    W0 = np.asarray(inputs["W0"], np.float32)
    b0 = np.asarray(inputs["b0"], np.float32)
    W1 = np.asarray(inputs["W1"], np.float32)
    b1 = np.asarray(inputs["b1"], np.float32)
    src = np.asarray(inputs["src"])
    dst = np.asarray(inputs["dst"])
    user_index = np.asarray(inputs["user_index"])
    item_index = np.asarray(inputs["item_index"])

    nn_, d = cfg["n_nodes"], cfg["d"]
    ncores = cfg["n_cores"]
    k = cfg["k"]
    npc, nt, rows = derived(cfg)

    order = np.argsort(dst, kind="stable")
    src_s = src[order].astype(np.int64)
    dst_s = dst[order].astype(np.int64)

    tile_lo_node, tile_hi_node = [], []
    for c in range(ncores):
        for t in range(nt):
            lo = c * npc + t * P
            hi = min(c * npc + npc, lo + P)
            tile_lo_node.append(lo)
            tile_hi_node.append(hi)
    los = np.searchsorted(dst_s, np.array(tile_lo_node))
    his = np.searchsorted(dst_s, np.array(tile_hi_node))
    cnts = his - los
    et = int(max(1, np.max((cnts + P - 1) // P)))  # uniform edge-tiles per dst-tile

    # padding edge slots gather row 0 (their one-hot row is all-zero)
    src_idx = np.zeros((ncores, P, nt * et), np.int32)
    dstloc = np.full((ncores, P, nt * et), 300.0, np.float32)
    for g in range(ncores * nt):
        c, t = divmod(g, nt)
        lo, hi = los[g], his[g]
        n = hi - lo
        if n == 0:
            continue
        e = np.arange(n)
        j = e // P
        p = e % P
        src_idx[c, p, t * et + j] = src_s[lo:hi]
        dstloc[c, p, t * et + j] = dst_s[lo:hi] - (c * npc + t * P)

    W0t = np.ascontiguousarray(W0.T)
    W1t = np.ascontiguousarray(W1.T)
    b0r = np.ascontiguousarray(np.broadcast_to(b0, (P, d))).astype(np.float32)
    b1r = np.ascontiguousarray(np.broadcast_to(b1, (P, d))).astype(np.float32)

    in_maps = []
    for c in range(ncores):
        embT = np.zeros((d, nt * P), np.float32)
        embT[:, :npc] = emb[c * npc:(c + 1) * npc].T
        # scoring: flatten this core's (row, k) pairs; column j holds pairs
        # [j*P, (j+1)*P) so each gather is one index per partition
        ui = user_index[c * rows:(c + 1) * rows].astype(np.int32).ravel()
        ii = item_index[c * rows:(c + 1) * rows].astype(np.int32).ravel()
        ui = np.ascontiguousarray(ui.reshape(rows * k // P, P).T)
        ii = np.ascontiguousarray(ii.reshape(rows * k // P, P).T)
        in_maps.append(dict(
            embT=embT,
            W0t=W0t, W1t=W1t, b0r=b0r, b1r=b1r,
            src_idx=src_idx[c], dstloc=dstloc[c],
            user_idx=ui, item_idx=ii,
        ))
    return in_maps, et


def build_nc(cfg, et):
    """Builds + compiles the Bass program. Returns nc."""
    import concourse.bass as bass
    import concourse.bacc as bacc
    import concourse.mybir as mybir
    import concourse.tile as tile
    from concourse.masks import make_identity

    f32 = mybir.dt.float32
    i32 = mybir.dt.int32

    nn_, d = cfg["n_nodes"], cfg["d"]
    dv = d + 1
    ncores = cfg["n_cores"]
    k = cfg["k"]
    npc, nt, rows = derived(cfg)
    groups = [list(range(ncores))]
    nsc = rows * k // P              # scoring gather columns (pairs / P)

    nc = bacc.Bacc(
        "TRN2",
        target_bir_lowering=False,
        debug=False,
        enable_asserts=False,
        num_devices=ncores,
    )

    # ---------------- I/O ----------------
    embT_d = nc.dram_tensor("embT", [d, nt * P], f32, kind="ExternalInput")
    W0t_d = nc.dram_tensor("W0t", [d, d], f32, kind="ExternalInput")
    W1t_d = nc.dram_tensor("W1t", [d, d], f32, kind="ExternalInput")
    b0r_d = nc.dram_tensor("b0r", [P, d], f32, kind="ExternalInput")
    b1r_d = nc.dram_tensor("b1r", [P, d], f32, kind="ExternalInput")
    sidx_d = nc.dram_tensor("src_idx", [P, nt * et], i32, kind="ExternalInput")
    dloc_d = nc.dram_tensor("dstloc", [P, nt * et], f32, kind="ExternalInput")
    uidx_d = nc.dram_tensor("user_idx", [P, nsc], i32, kind="ExternalInput")
    iidx_d = nc.dram_tensor("item_idx", [P, nsc], i32, kind="ExternalInput")
    score_d = nc.dram_tensor("score", [P, nsc], f32, kind="ExternalOutput")

    g0_loc = nc.dram_tensor("g0_loc", [npc, dv], f32)
    g1_loc = nc.dram_tensor("g1_loc", [npc, dv], f32)
    h1_loc = nc.dram_tensor("h1_loc", [npc, d], f32)
    g0_full = nc.dram_tensor("g0_full", [nn_, dv], f32, addr_space="Shared")
    g1_full = nc.dram_tensor("g1_full", [nn_, dv], f32, addr_space="Shared")
    h1_full = nc.dram_tensor("h1_full", [nn_, d], f32, addr_space="Shared")

    with tile.TileContext(nc) as tc:
        with (
            tc.tile_pool(name="const", bufs=1) as cpool,
            tc.tile_pool(name="gath", bufs=48) as gpool,
            tc.tile_pool(name="oh", bufs=2) as ohpool,
            tc.tile_pool(name="work", bufs=3) as wpool,
            tc.tile_pool(name="stage", bufs=3) as spool,
            tc.tile_pool(name="sco", bufs=1) as scpool,
            tc.tile_pool(name="pacc", bufs=2, space="PSUM") as pacc,
            tc.tile_pool(name="ptp", bufs=2, space="PSUM") as ptp,
            tc.tile_pool(name="pg", bufs=2, space="PSUM") as pg,
        ):
            # ---------- constants ----------
            ident = cpool.tile([P, P], f32)
            make_identity(nc, ident[:])
            iota_i = cpool.tile([P, P], i32)
            nc.gpsimd.iota(iota_i[:], pattern=[[1, P]], base=0, channel_multiplier=0)
            iota_f = cpool.tile([P, P], f32)
            nc.vector.tensor_copy(iota_f[:], iota_i[:])

            embT_sb = cpool.tile([d, nt * P], f32)
            nc.sync.dma_start(embT_sb[:], embT_d[:, :])
            W0t_sb = cpool.tile([d, d], f32)
            nc.sync.dma_start(W0t_sb[:], W0t_d[:, :])
            W1t_sb = cpool.tile([d, d], f32)
            nc.sync.dma_start(W1t_sb[:], W1t_d[:, :])
            b0_sb = cpool.tile([P, d], f32)
            nc.sync.dma_start(b0_sb[:], b0r_d[:, :])
            b1_sb = cpool.tile([P, d], f32)
            nc.sync.dma_start(b1_sb[:], b1r_d[:, :])
            sidx_sb = cpool.tile([P, nt * et], i32)
            nc.sync.dma_start(sidx_sb[:], sidx_d[:, :])
            dloc_sb = cpool.tile([P, nt * et], f32)
            nc.sync.dma_start(dloc_sb[:], dloc_d[:, :])
            uidx_sb = cpool.tile([P, nsc], i32)
            nc.sync.dma_start(uidx_sb[:], uidx_d[:, :])
            iidx_sb = cpool.tile([P, nsc], i32)
            nc.sync.dma_start(iidx_sb[:], iidx_d[:, :])

            def stage_g(gsrc_psum, dest_dram, t):
                gst = spool.tile([P, dv], f32, tag="gstage")
                nc.vector.tensor_copy(gst[:, :d], gsrc_psum[:])
                nc.vector.memset(gst[:, d:dv], 1.0)
                r = min(P, npc - t * P)
                nc.sync.dma_start(dest_dram[t * P:t * P + r, :], gst[:r, :])

            # ---------- g0 = emb @ W0.T ----------
            for t in range(nt):
                g0p = pg.tile([P, d], f32, tag="pg")
                nc.tensor.matmul(
                    g0p[:], lhsT=embT_sb[:, t * P:(t + 1) * P], rhs=W0t_sb[:],
                    start=True, stop=True,
                )
                stage_g(g0p, g0_loc, t)

            nc.gpsimd.collective_compute(
                "AllGather", mybir.AluOpType.bypass, replica_groups=groups,
                ins=[g0_loc[:, :]], outs=[g0_full[:, :]],
            )

            # ---------- layers ----------
            kb = cfg["kb"]

            def layer(gtab, b_sb, is_last):
                for t in range(nt):
                    gaths = []
                    for j in range(et):
                        gt = gpool.tile([P, dv], f32, tag="gath")
                        nc.gpsimd.indirect_dma_start(
                            out=gt[:, :],
                            out_offset=None,
                            in_=gtab[:, :],
                            in_offset=bass.IndirectOffsetOnAxis(
                                ap=sidx_sb[:, t * et + j:t * et + j + 1], axis=0,
                            ),
                        )
                        gaths.append(gt)
                    oh = ohpool.tile([P, et, P], f32, tag="oh")
                    for j0 in range(0, et, kb):
                        jn = min(kb, et - j0)
                        nc.vector.tensor_tensor(
                            out=oh[:, j0:j0 + jn, :],
                            in0=iota_f[:, None, :].broadcast_to([P, jn, P]),
                            in1=dloc_sb[:, t * et + j0:t * et + j0 + jn][:, :, None]
                                .broadcast_to([P, jn, P]),
                            op=mybir.AluOpType.is_equal,
                        )
                    acc = pacc.tile([P, dv], f32, tag="pacc")
                    for j in range(et):
                        nc.tensor.matmul(
                            acc[:], lhsT=oh[:, j, :], rhs=gaths[j][:, :],
                            start=(j == 0), stop=(j == et - 1),
                        )
                    # mean, bias, tanh
                    degc = wpool.tile([P, 1], f32, tag="degc")
                    nc.vector.tensor_scalar_max(degc[:], acc[:, d:dv], 1.0)
                    recip = wpool.tile([P, 1], f32, tag="recip")
                    nc.vector.reciprocal(recip[:], degc[:])
                    mean = wpool.tile([P, d], f32, tag="mean")
                    nc.scalar.activation(
                        mean[:], acc[:, :d],
                        mybir.ActivationFunctionType.Copy, scale=recip[:],
                    )
                    z = wpool.tile([P, d], f32, tag="z")
                    nc.vector.tensor_add(z[:], mean[:], b_sb[:])
                    h = wpool.tile([P, d], f32, tag="h")
                    nc.scalar.activation(h[:], z[:], mybir.ActivationFunctionType.Tanh)
                    r = min(P, npc - t * P)
                    if is_last:
                        nc.sync.dma_start(h1_loc[t * P:t * P + r, :], h[:r, :])
                    else:
                        tp = ptp.tile([d, P], f32, tag="ptp")
                        nc.tensor.transpose(tp[:], h[:], ident[:])
                        hT = wpool.tile([d, P], f32, tag="hT")
                        nc.vector.tensor_copy(hT[:], tp[:])
                        gm = pg.tile([P, d], f32, tag="pg")
                        nc.tensor.matmul(gm[:], lhsT=hT[:], rhs=W1t_sb[:],
                                         start=True, stop=True)
                        stage_g(gm, g1_loc, t)

            layer(g0_full, b0_sb, is_last=False)
            nc.gpsimd.collective_compute(
                "AllGather", mybir.AluOpType.bypass, replica_groups=groups,
                ins=[g1_loc[:, :]], outs=[g1_full[:, :]],
            )
            layer(g1_full, b1_sb, is_last=True)
            nc.gpsimd.collective_compute(
                "AllGather", mybir.AluOpType.bypass, replica_groups=groups,
                ins=[h1_loc[:, :]], outs=[h1_full[:, :]],
            )

            # ---------- scoring ----------
            kcn = cfg["kc"] * rows // P          # gather columns per chunk
            sc_out = cpool.tile([P, nsc], f32)
            for c0 in range(0, nsc, kcn):
                cn = min(kcn, nsc - c0)
                ug = scpool.tile([P, kcn, d], f32, tag="ug")
                ig = scpool.tile([P, kcn, d], f32, tag="ig")
                for j in range(cn):
                    nc.gpsimd.indirect_dma_start(
                        out=ug[:, j, :], out_offset=None, in_=h1_full[:, :],
                        in_offset=bass.IndirectOffsetOnAxis(
                            ap=uidx_sb[:, c0 + j:c0 + j + 1], axis=0),
                    )
                    nc.gpsimd.indirect_dma_start(
                        out=ig[:, j, :], out_offset=None, in_=h1_full[:, :],
                        in_offset=bass.IndirectOffsetOnAxis(
                            ap=iidx_sb[:, c0 + j:c0 + j + 1], axis=0),
                    )
                prod = scpool.tile([P, kcn, d], f32, tag="prod")
                nc.vector.tensor_tensor(
                    out=prod[:, :cn, :], in0=ug[:, :cn, :], in1=ig[:, :cn, :],
                    op=mybir.AluOpType.mult,
                )
                nc.vector.tensor_reduce(
                    out=sc_out[:, c0:c0 + cn], in_=prod[:, :cn, :],
                    axis=mybir.AxisListType.X, op=mybir.AluOpType.add,
                )
            nc.sync.dma_start(score_d[:, :], sc_out[:])

    nc.compile()
    return nc


_CACHE = {}


def _get_nc(cfg_key, cfg, et):
    key = (cfg_key, et)
    if key not in _CACHE:
        _CACHE[key] = build_nc(cfg, et)
    return _CACHE[key]


def assemble_score(results, cfg):
    npc, nt, rows = derived(cfg)
    k = cfg["k"]
    parts = []
    for r in results:
        sc = r["score"]                    # [P, nsc]; column j = pairs [j*P,(j+1)*P)
        parts.append(np.ascontiguousarray(sc.T).reshape(rows, k))
    return np.concatenate(parts, axis=0).astype(np.float32)


def run(inputs, cfg=None, trace=False):
    """Returns (score [batch, k] float32, BassKernelResults)."""
    from concourse.bass_utils import run_bass_kernel_spmd

    cfg = dict(DEFAULT_CFG, **(cfg or {}))
    in_maps, et = preprocess(inputs, cfg)
    cfg_key = tuple(sorted((kk, v) for kk, v in cfg.items()))
    nc = _get_nc(cfg_key, cfg, et)
    res = run_bass_kernel_spmd(
        nc, in_maps, core_ids=list(range(cfg["n_cores"])), trace=trace,
    )
    return assemble_score(res.results, cfg), res


def kernel(**inputs) -> np.ndarray:
    score, _ = run(inputs)
    return score
